# revision 1
# baseline (speedup 1.0000x reference)
"""Trainium2 Bass kernel for nn_MultiHeadedAttention_88021059764737.

Math (reference):
  q = Wq @ query + bq   (per batch; [D, N]), same k, v
  per head h (DIM=64): scores = q_h^T k_h / 8   [N, N]
  adj = dense adjacency counts from edges        [N, N]
  prob = exp(scores * adj) / rowsum
  x_h = v_h @ prob^T ; out = Wm @ x + bm

Device design (per core; 8 cores = 4 batches x 2 n-halves):
  - delta decomposition: exp(s*adj) = 1 + adj*(exp(s)-1); the "+1" part
    is restored by an analytically-exact vsum (Wv @ rowsum(value) + N*bv).
  - adj*(exp(s)-1) = adj*exp(s) - adj is split engine-wise:
      * ACT: e = exp(s) (bf16, from QK PSUM, the only dense ACT pass)
      * DVE: ed = e * adj (one tensor_tensor at 2x bf16 mode)
      * PE:  PV_e = vT @ ed (bf16) and PV_a = vT8 @ (-adj8) via fp8
        DoubleRow matmuls accumulated into the same PSUM banks; the ones
        columns of vT/vT8 accumulate the softmax denominator row.
  - QK and all projections run as fp8e4m3 DoubleRow matmuls (0.5
    cycles/col): weights/input activations are host-quantized (x64 scale
    folded out via the ACT exp scale = 2^-12) and fold to the
    [32p, 2-slot] layout via SBUF->DRAM->SBUF bounce DMAs.
  - normalization is fused: xsb = (x + vsum) / denom on Pool (stt with
    op1=divide, denom broadcast), out-projection in exact f32r, bm bias
    folded into the Pool PSUM->SBUF staging copy.
"""

import os
import sys

sys.path.insert(0, "/opt/trn_rl_repo")

import numpy as np
import ml_dtypes

B, D, H, N, E = 4, 256, 4, 2048, 32768
DIM = D // H  # 64
NCORES = 8
NH = N // 2  # per-core n-half
SCALE = 1.0 / np.sqrt(np.float32(DIM))  # 1/8
ALPHA = 64.0  # fp8 weight scale; exp applies 1/ALPHA^2


def build_nc(N_=N, NH_=NH):
    import concourse.bass as bass  # noqa: F401
    import concourse.mybir as mybir
    import concourse.tile as tile
    from concourse import bacc

    f32 = mybir.dt.float32
    f32r = mybir.dt.float32r
    bf16 = mybir.dt.bfloat16
    fp8 = mybir.dt.float8e4
    MT = N_ // 128          # m tiles of 128
    NCH = NH_ // 512        # n chunks of 512
    MG = MT // 4            # adj groups of 4 m-tiles
    Exp = mybir.ActivationFunctionType.Exp
    mult = mybir.AluOpType.mult
    add = mybir.AluOpType.add
    div = mybir.AluOpType.divide
    DR = mybir.MatmulPerfMode.DoubleRow

    nc = bacc.Bacc()
    # ---- DRAM I/O ----
    xq8 = nc.dram_tensor("xq8", [128, 2, NH_], fp8, kind="ExternalInput")
    xk8 = nc.dram_tensor("xk8", [128, 2, N_], fp8, kind="ExternalInput")
    xv8 = nc.dram_tensor("xv8", [128, 2, N_], fp8, kind="ExternalInput")
    xv32 = nc.dram_tensor("xv32", [128, 2, N_], f32, kind="ExternalInput")
    w8pack = nc.dram_tensor("w8pack", [128, 2, 3 * D], fp8,
                            kind="ExternalInput")  # [di, slot, wq|wk|wv]
    wm32 = nc.dram_tensor("wm32", [128, 2, D], f32r, kind="ExternalInput")
    wv32 = nc.dram_tensor("wv32", [128, 2, D], f32, kind="ExternalInput")
    bcol = nc.dram_tensor("bcol", [128, 6], f32,
                          kind="ExternalInput")  # bq8(2)|bk8(2)|bm(2)
    brow = nc.dram_tensor("brow", [1, 2 * D], f32,
                          kind="ExternalInput")  # a*bv | bv
    brow16 = nc.dram_tensor("brow16", [1, D], bf16,
                            kind="ExternalInput")  # a*bv in bf16
    adjT = nc.dram_tensor("adjT", [N_, NH_], bf16, kind="ExternalInput")
    adjT8n = nc.dram_tensor("adjT8n", [64, 2, MT, NH_], fp8,
                            kind="ExternalInput")  # -adj counts, m-folded
    # DRAM scratch for partition-fold bounces
    qf8d = nc.dram_tensor("qf8d", [128, 2, NH_], fp8, kind="Internal")
    kf8d = nc.dram_tensor("kf8d", [128, 2, N_], fp8, kind="Internal")
    vt8d = nc.dram_tensor("vt8d", [128, MT, 4 * 65], fp8, kind="Internal")
    out = nc.dram_tensor("out", [D, NH_], f32, kind="ExternalOutput")
    KDBG = bool(int(os.environ.get("KDBG", "0")))
    if KDBG:
        dbg_vsum = nc.dram_tensor("dbg_vsum", [128, 4], f32,
                                  kind="ExternalOutput")
        dbg_xsb = nc.dram_tensor("dbg_xsb", [128, 2, 512], f32,
                                 kind="ExternalOutput")
        dbg_den = nc.dram_tensor("dbg_den", [1, 4, 512], f32,
                                 kind="ExternalOutput")
        dbg_ed = nc.dram_tensor("dbg_ed", [128, 4, 512], bf16,
                                kind="ExternalOutput")
        dbg_vt = nc.dram_tensor("dbg_vt", [128, MT, 4 * 65], bf16,
                                kind="ExternalOutput")
        dbg_qf = nc.dram_tensor("dbg_qf", [128, 2, NH_], fp8,
                                kind="ExternalOutput")
        dbg_kf = nc.dram_tensor("dbg_kf", [128, 2, N_], fp8,
                                kind="ExternalOutput")
        dbg_denb = nc.dram_tensor("dbg_denb", [128, 4, 512], f32,
                                  kind="ExternalOutput")

    from contextlib import ExitStack

    with tile.TileContext(nc) as tc, ExitStack() as ctx:
        consts = ctx.enter_context(tc.tile_pool(name="consts", bufs=1))
        big = ctx.enter_context(tc.tile_pool(name="big", bufs=1))
        work = ctx.enter_context(tc.tile_pool(name="work", bufs=2))
        pacc = ctx.enter_context(tc.tile_pool(name="pacc", bufs=1, space="PSUM"))
        psc = ctx.enter_context(tc.tile_pool(name="psc", bufs=2, space="PSUM"))

        # ---- constants (SP queue; w8+bcol first: k-proj gates all) ----
        w8_sb = consts.tile([128, 2, 3 * D], fp8, tag="w8")
        wm_sb = consts.tile([128, 2, D], f32r, tag="wm")
        wv_sb = consts.tile([128, 2, D], f32, tag="wv")
        bcol_sb = consts.tile([128, 6], f32, tag="bcol")
        brow_sb = consts.tile([1, 2 * D], f32, tag="brow")
        brow16_sb = consts.tile([1, D], bf16, tag="brow16")
        ones_sb = consts.tile([1, 512], f32, tag="ones")
        ones16_sb = consts.tile([1, 128], bf16, tag="ones16")
        nc.sync.dma_start(out=w8_sb[:], in_=w8pack[:])
        nc.vector.memset(ones_sb[:], 1.0)
        nc.vector.memset(ones16_sb[:], 1.0)
        w8q = w8_sb[:, :, 0 * D:1 * D]
        w8k = w8_sb[:, :, 1 * D:2 * D]
        w8v = w8_sb[:, :, 2 * D:3 * D]

        # ---- inputs (projection inputs first: they gate everything) ----
        xk8_sb = big.tile([128, 2, N_], fp8, tag="xk8")
        xq8_sb = big.tile([128, 2, NH_], fp8, tag="xq8")
        xv8_sb = big.tile([128, 2, N_], fp8, tag="xv8")
        xv_sb = [big.tile([128, 1, N_], f32, tag=f"xv32_{kc}", name=f"xv32_{kc}")
                 for kc in range(2)]
        nc.sync.dma_start(out=xk8_sb[:], in_=xk8[:])
        nc.sync.dma_start(out=bcol_sb[:], in_=bcol[:])
        nc.sync.dma_start(out=xq8_sb[:], in_=xq8[:])
        nc.sync.dma_start(out=brow_sb[:], in_=brow[:])
        nc.sync.dma_start(out=brow16_sb[:], in_=brow16[:])
        nc.sync.dma_start(out=xv8_sb[:], in_=xv8[:])
        nc.sync.dma_start(out=wm_sb[:], in_=wm32[:])
        nc.sync.dma_start(out=wv_sb[:], in_=wv32[:])

        # adjacency: 8 bf16 groups of 2 m-tiles, loaded per n-chunk half so
        # chunk-0 columns arrive first; adj8 (fp8, m-folded) likewise.
        # Group 0 + adj8[0:4] go on the ACT queue (idle early); the rest are
        # staged on SP by deadline.
        MG2 = MT // 2
        adj_g = [None] * MG2
        adj8_sb = big.tile([64, 2, MT, NH_], fp8, tag="adj8")

        def adj_tile(g, tag=None):
            adj_g[g] = big.tile([128, 2, NH_], bf16, tag=tag or f"adjE{g}",
                                name=f"adj{g}")

        def adj_load(g, eng, half=None):
            sl = slice(0, NH_) if half is None else \
                slice(512 * half, 512 * (half + 1))
            eng.dma_start(
                out=adj_g[g][:, :, sl],
                in_=adjT[256 * g:256 * (g + 1), sl].rearrange(
                    "(mt p) n -> p mt n", p=128),
            )

        def adj8_load(eng, mlo, mhi, half):
            sl = slice(512 * half, 512 * (half + 1))
            for s in range(2):
                eng.dma_start(out=adj8_sb[:, s, mlo:mhi, sl],
                              in_=adjT8n[:, s, mlo:mhi, sl])

        adj_tile(0)
        adj_load(0, nc.scalar)
        adj8_load(nc.scalar, 0, 4, 0)

        # ---- k projection (fp8 DoubleRow) -> copy (+bias, fp8) -> bounce --
        qf8_sb = big.tile([128, 2, NH_], fp8, tag="qf8")
        kf8_sb = big.tile([128, 2, N_], fp8, tag="kf8")
        k8_sb = big.tile([32, 2, 2, 2, N_], fp8, tag="k8")
        q8_sb = big.tile([32, 2, 2, 2, NH_], fp8, tag="q8")
        Ident = mybir.ActivationFunctionType.Identity

        def fold_bounce(fsb, fd, dst, mtile):
            # [h2*64+s*32+p, mtile] -> [p, h2, s, mtile] via DRAM
            nc.sync.dma_start(out=fd[:, mtile, :], in_=fsb[:, mtile, :])
            for h2 in range(2):
                nc.sync.dma_start(
                    out=dst[:, h2, :, mtile, :],
                    in_=fd[64 * h2:64 * h2 + 64, mtile, :].rearrange(
                        "(s p) n -> p s n", s=2, p=32))

        for mtile in range(2):
            for np2 in range(N_ // 1024):
                ps = psc.tile([128, 1024], f32, tag="sc")
                for j in range(2):
                    ncol = np2 * 2 + j
                    nc.tensor.matmul(
                        ps[:, 512 * j:512 * (j + 1)],
                        lhsT=w8k[:, :, 128 * mtile:128 * (mtile + 1)],
                        rhs=xk8_sb[:, :, 512 * ncol:512 * (ncol + 1)],
                        start=True, stop=True, perf_mode=DR,
                    )
                if mtile == 0:
                    nc.scalar.activation(
                        out=kf8_sb[:, mtile, 1024 * np2:1024 * (np2 + 1)],
                        in_=ps[:], func=Ident,
                        bias=bcol_sb[:, 2 + mtile:3 + mtile], scale=1.0,
                    )
                else:
                    nc.vector.tensor_scalar(
                        out=kf8_sb[:, mtile, 1024 * np2:1024 * (np2 + 1)],
                        in0=ps[:], scalar1=bcol_sb[:, 2 + mtile:3 + mtile],
                        scalar2=None, op0=add,
                    )
            fold_bounce(kf8_sb, kf8d, k8_sb, mtile)

        # ---- q projection (DVE copies -> early psc release) -> bounce ----
        for mtile in range(2):
            ps = psc.tile([128, 1024], f32, tag="sc")
            for ncol in range(NH_ // 512):
                nc.tensor.matmul(
                    ps[:, 512 * ncol:512 * (ncol + 1)],
                    lhsT=w8q[:, :, 128 * mtile:128 * (mtile + 1)],
                    rhs=xq8_sb[:, :, 512 * ncol:512 * (ncol + 1)],
                    start=True, stop=True, perf_mode=DR,
                )
            nc.vector.tensor_scalar(
                out=qf8_sb[:, mtile, :], in0=ps[:],
                scalar1=bcol_sb[:, mtile:mtile + 1], scalar2=None, op0=add,
            )
            fold_bounce(qf8_sb, qf8d, q8_sb, mtile)

        # urgent chunk-0 adjacency halves (dedicated slots + dead xk8 slot)
        for g in range(1, MG2):
            adj_tile(g, tag=("xk8" if g == 5 else "xv8" if g == 6 else
                             "kf8" if g == 7 else None))
        for g in range(1, MG2):
            adj_load(g, nc.sync, half=0)

        # ---- v projection (fp8 DR + bf16 bias matmul) -> vT bf16 ----
        # Runs between k and q so the q-proj psc slots (released early by
        # the DVE qf8 copies) are what gate the attention QK stream.
        # vT8 (fp8, m-folded, for PV_a) derives from vT via a casting
        # gpsimd DMA to DRAM, then a folding DMA back.
        vT_sb = big.tile([128, MT, 4 * 65], bf16, tag="vT")
        vT8_sb = big.tile([64, 2, MT, 4 * 65], fp8, tag="vT8")
        nc.vector.memset(
            vT_sb.rearrange("q mt (h e) -> q mt h e", e=65)[:, :, :, 64:65], 1.0)
        def emit_vproj_group(mtg, on_act):
            ps = psc.tile([128, 1024], f32, tag="sc")
            for j in range(4):
                mt = mtg * 4 + j
                nc.tensor.matmul(
                    ps[:, 256 * j:256 * (j + 1)],
                    lhsT=xv8_sb[:, :, 128 * mt:128 * (mt + 1)],
                    rhs=w8v[:], start=True, stop=False, perf_mode=DR,
                )
                nc.tensor.matmul(  # + ALPHA*bv via K=1 ones row (bf16 rate)
                    ps[:, 256 * j:256 * (j + 1)],
                    lhsT=ones16_sb[0:1, 0:128], rhs=brow16_sb[0:1, 0:D],
                    start=False, stop=True,
                )
            # batched psum->vT copy for the group
            vout = vT_sb.rearrange("q mt (h e) -> q mt h e", e=65)[
                :, 4 * mtg:4 * (mtg + 1), :, 0:64]
            vin = ps[:].rearrange("q (mtl h d) -> q mtl h d", h=4, d=64)
            if on_act:
                nc.scalar.activation(out=vout, in_=vin, func=Ident,
                                     scale=1.0 / ALPHA)
            else:
                nc.vector.tensor_scalar(out=vout, in0=vin,
                                        scalar1=1.0 / ALPHA, scalar2=None,
                                        op0=mult)
            # fold to fp8 [64, 2slot, mt, 260] via DRAM (casting gpsimd leg)
            msl = slice(4 * mtg, 4 * (mtg + 1))
            nc.gpsimd.dma_start(out=vt8d[:, msl, :], in_=vT_sb[:, msl, :])
            nc.sync.dma_start(
                out=vT8_sb[:, :, msl, :],
                in_=vt8d[:, msl, :].rearrange("(s p) mt e -> p s mt e",
                                              s=2, p=64))
            if mtg == 0:
                adj8_load(nc.sync, 4, 10, 0)
            elif mtg == 1:
                adj8_load(nc.sync, 10, MT, 0)

        for mtg in range(4):
            emit_vproj_group(mtg, on_act=True)

        # chunk-1 column halves (deadlines are far out)
        for g in range(1, MG2):
            adj_load(g, nc.sync, half=1)
        adj8_load(nc.sync, 0, 8, 1)
        adj8_load(nc.sync, 8, MT, 1)

        # xv32 arrives late (only feeds the analytic vsum)
        for kc in range(2):
            nc.sync.dma_start(out=xv_sb[kc][:], in_=xv32[:, kc:kc + 1, :])
        vv_sb = consts.tile([128, 2], f32, tag="vv")
        vsum_sb = consts.tile([128, 4], f32, tag="vsum")
        nconst = consts.tile([1, 1], f32, tag="nconst")
        nc.vector.memset(nconst[:], float(N_))

        def emit_vsum():
            # vsum[dd] = Wv_perm @ (sum_m value) + N*bv (exact f32);
            # emitted after chunk 0's mt-loop so the DVE/PE queues never
            # block on the late xv32 arrival.
            for kc in range(2):
                nc.vector.tensor_reduce(
                    out=vv_sb[:, kc:kc + 1], in_=xv_sb[kc][:, 0, :],
                    axis=mybir.AxisListType.X, op=add)
            vs_ps = psc.tile([128, 1024], f32, tag="sc", name="vs_ps")
            for mtile in range(2):
                for kc in range(2):
                    nc.tensor.matmul(
                        vs_ps[0:128, mtile:mtile + 1],
                        lhsT=wv_sb[:, kc, 128 * mtile:128 * (mtile + 1)],
                        rhs=vv_sb[:, kc:kc + 1],
                        start=(kc == 0), stop=False,
                    )
                nc.tensor.matmul(  # + N * bv
                    vs_ps[0:128, mtile:mtile + 1],
                    lhsT=brow_sb[0:1, D + 128 * mtile:D + 128 * (mtile + 1)],
                    rhs=nconst[:],
                    start=False, stop=True,
                )
            for hh in range(4):
                sl = vs_ps[64 * (hh % 2):64 * (hh % 2) + 64,
                           hh // 2:hh // 2 + 1]
                nc.vector.tensor_copy(out=vsum_sb[0:64, hh:hh + 1], in_=sl)
                nc.vector.tensor_copy(out=vsum_sb[64:128, hh:hh + 1], in_=sl)

        # ---- attention (epilogues software-pipelined into the next chunk) --
        x_h_of = {}
        xsb_of = {}
        dumps = {}

        def emit_mt(c, mt):
            nsl = slice(512 * c, 512 * (c + 1))
            x_h = x_h_of[c]
            adjtile = adj_g[mt // 2]
            adj_sl = adjtile[:, mt % 2, nsl]
            adj_b = bass.AP(tensor=adj_sl.tensor, offset=adj_sl.offset,
                            ap=[adj_sl.ap[0], [0, 4], adj_sl.ap[-1]])
            e_t = work.tile([128, 4, 512], bf16, tag="exp", name="e_t",
                            bufs=5)
            for p in range(2):
                scp = psc.tile([128, 1024], f32, tag="sc", name="scp")
                for h2 in range(2):
                    h = 2 * p + h2
                    nc.tensor.matmul(
                        scp[:, 512 * h2:512 * (h2 + 1)],
                        lhsT=k8_sb[:, h % 2, :, h // 2,
                                   128 * mt:128 * (mt + 1)],
                        rhs=q8_sb[:, h % 2, :, h // 2, nsl],
                        start=True, stop=True, perf_mode=DR,
                    )
                nc.scalar.activation(
                    out=e_t[:, 2 * p:2 * p + 2, :], in_=scp[:],
                    func=Exp, scale=1.0 / (ALPHA * ALPHA),
                )
            ed_t = work.tile([128, 4, 512], bf16, tag="dlt", name="ed_t",
                             bufs=4)
            nc.vector.tensor_tensor(out=ed_t[:], in0=e_t[:], in1=adj_b,
                                    op=mult)
            dumps["ed"] = ed_t
            for hh in range(4):
                nc.tensor.matmul(
                    x_h[hh][0:65, :],
                    lhsT=vT_sb[:, mt, 65 * hh:65 * (hh + 1)],
                    rhs=ed_t[:, hh, :],
                    start=(mt == 0), stop=False,
                )
                nc.tensor.matmul(  # -= vT @ adj  (fp8 DoubleRow)
                    x_h[hh][0:65, :],
                    lhsT=vT8_sb[:, :, mt, 65 * hh:65 * (hh + 1)],
                    rhs=adj8_sb[:, :, mt, nsl],
                    start=False, stop=(mt == MT - 1), perf_mode=DR,
                )

        norm_tiles = {}

        def emit_norm(c, heads=range(4)):
            # xsb = (x + vsum) / (row64 + N)
            x_h = x_h_of[c]
            if c not in norm_tiles:
                norm_tiles[c] = (
                    work.tile([1, 4, 512], f32, tag="den", name="den",
                              bufs=1),
                    work.tile([128, 4, 512], f32, tag="denb", name="den_b",
                              bufs=1),
                    work.tile([128, 2, 512], f32r, tag="xsb", name="xsb"),
                )
            denom_t, den_b, xsb = norm_tiles[c]
            xsb_of[c] = xsb
            dumps["den"] = denom_t
            dumps["denb"] = den_b
            for hh in heads:
                h2, kc = hh % 2, hh // 2
                nc.scalar.activation(  # row64 + N
                    out=denom_t[0:1, hh, :], in_=x_h[hh][64:65, :],
                    func=Ident, bias=nconst[0:1, 0:1], scale=1.0)
                nc.vector.reciprocal(out=denom_t[0:1, hh, :],
                                     in_=denom_t[0:1, hh, :])
                nc.gpsimd.partition_broadcast(
                    den_b[:, hh, :], denom_t[0:1, hh, :])
                nc.vector.scalar_tensor_tensor(
                    out=xsb[64 * h2:64 * h2 + 64, kc, :],
                    in0=x_h[hh][0:64, :],
                    scalar=vsum_sb[64 * h2:64 * h2 + 64, hh:hh + 1],
                    in1=den_b[64 * h2:64 * h2 + 64, hh, :],
                    op0=add, op1=mult,
                )

        def emit_outproj(c):
            nsl = slice(512 * c, 512 * (c + 1))
            xsb = xsb_of[c]
            op_ps = psc.tile([128, 1024], f32, tag="sc", name="op_ps")
            out_t = work.tile([128, 1024], f32, tag="osb", name="out_t")
            for kc in range(2):
                for mtile in range(2):
                    nc.tensor.matmul(
                        op_ps[:, 512 * mtile:512 * (mtile + 1)],
                        lhsT=wm_sb[:, kc, 128 * mtile:128 * (mtile + 1)],
                        rhs=xsb[:, kc, :],
                        start=(kc == 0), stop=(kc == 1),
                    )
            for mtile in range(2):
                nc.vector.tensor_scalar(
                    out=out_t[:, 512 * mtile:512 * (mtile + 1)],
                    in0=op_ps[:, 512 * mtile:512 * (mtile + 1)],
                    scalar1=bcol_sb[:, 4 + mtile:5 + mtile],
                    scalar2=None, op0=add,
                )
                nc.sync.dma_start(
                    out=out[128 * mtile:128 * (mtile + 1), nsl],
                    in_=out_t[:, 512 * mtile:512 * (mtile + 1)])

        for c in range(NCH):
            x_h_of[c] = [pacc.tile([128, 512], f32, tag=f"x{hh}",
                                   name=f"x{hh}") for hh in range(4)]
            for mt in range(MT):
                if c > 0 and mt == 3:
                    emit_norm(c - 1)       # previous chunk's normalize
                if c > 0 and mt == 14:
                    emit_outproj(c - 1)    # previous chunk's projection
                emit_mt(c, mt)
            if c == 0:
                emit_vsum()
        emit_norm(NCH - 1)
        emit_outproj(NCH - 1)

        if KDBG:
            nc.sync.dma_start(out=dbg_vsum[:], in_=vsum_sb[:])
            nc.sync.dma_start(out=dbg_xsb[:],
                              in_=xsb_of[NCH - 1][:].bitcast(f32))
            nc.sync.dma_start(out=dbg_qf[:], in_=qf8_sb[:])
            nc.sync.dma_start(out=dbg_kf[:], in_=kf8_sb[:])
            nc.sync.dma_start(out=dbg_vt[:], in_=vT_sb[:])
            nc.sync.dma_start(out=dbg_ed[:], in_=dumps["ed"][:])
            nc.sync.dma_start(out=dbg_den[:], in_=dumps["den"][:])
            nc.sync.dma_start(out=dbg_denb[:], in_=dumps["denb"][:])

    nc.compile()
    return nc


def host_prep(query, key, value, edges, Wq, bq, Wk, bk, Wv, bv, Wm, bm,
              N_=N, NH_=NH, B_=B):
    """Returns per-core input maps."""
    f32 = np.float32
    fp8 = ml_dtypes.float8_e4m3
    query = np.asarray(query, f32)
    key = np.asarray(key, f32)
    value = np.asarray(value, f32)
    edges = np.asarray(edges)
    Wq, bq = np.asarray(Wq, f32), np.asarray(bq, f32)
    Wk, bk = np.asarray(Wk, f32), np.asarray(bk, f32)
    Wv, bv = np.asarray(Wv, f32), np.asarray(bv, f32)
    Wm, bm = np.asarray(Wm, f32), np.asarray(bm, f32)
    MT = N_ // 128

    # head-major permutation: dd = h*DIM + dl  <->  o = dl*H + h
    dd = np.arange(D)
    perm = (dd % DIM) * H + (dd // DIM)

    def w8_layout(WT):  # WT [256(K=d_in), 256(dd)] -> [128, 2, 256] fp8
        return np.ascontiguousarray(
            WT.reshape(2, 128, D).transpose(1, 0, 2)).astype(fp8)

    def w32_layout(WT):
        return np.ascontiguousarray(WT.reshape(2, 128, D).transpose(1, 0, 2))

    wq8 = w8_layout((Wq[perm, :] * (ALPHA * SCALE)).T)
    wk8 = w8_layout((Wk[perm, :] * ALPHA).T)
    wv8 = w8_layout((Wv[perm, :] * ALPHA).T)
    w8pack_dev = np.ascontiguousarray(np.concatenate([wq8, wk8, wv8], axis=2))
    wm_dev = w32_layout(Wm[:, perm].T)
    wv32_dev = w32_layout(Wv[perm, :].T)
    bq8 = np.ascontiguousarray((bq[perm] * (ALPHA * SCALE)).reshape(2, 128).T)
    bk8 = np.ascontiguousarray((bk[perm] * ALPHA).reshape(2, 128).T)
    bm_col = np.ascontiguousarray(bm.reshape(2, 128).T)
    bcol_dev = np.ascontiguousarray(
        np.concatenate([bq8, bk8, bm_col], axis=1))
    brow_dev = np.ascontiguousarray(
        np.concatenate([bv[perm] * ALPHA, bv[perm]]).reshape(1, 2 * D))
    brow16_dev = np.ascontiguousarray(
        (bv[perm] * ALPHA).reshape(1, D)).astype(ml_dtypes.bfloat16)

    def fold_x(x):  # [256, n] -> [128, 2, n] fp8
        return np.ascontiguousarray(
            x.reshape(2, 128, x.shape[1]).transpose(1, 0, 2)).astype(fp8)

    in_maps = []
    ncores = 2 * B_
    for c in range(ncores):
        b, half = c // 2, c % 2
        ns = slice(half * NH_, (half + 1) * NH_)
        adj = np.zeros((N_, N_), f32)
        np.add.at(adj, (edges[b, 0].astype(np.int64),
                        edges[b, 1].astype(np.int64)), 1.0)
        adjT_c = np.ascontiguousarray(adj[ns, :].T)
        # fp8 negated, m-folded: [64, 2slot, mt, n]
        adjT8n_c = np.ascontiguousarray(
            (-adjT_c).reshape(MT, 2, 64, NH_).transpose(2, 1, 0, 3)
        ).astype(fp8)
        in_maps.append({
            "xq8": fold_x(query[b][:, ns]),
            "xk8": fold_x(key[b]),
            "xv8": fold_x(value[b]),
            "xv32": np.ascontiguousarray(
                value[b].reshape(2, 128, N_).transpose(1, 0, 2)),
            "w8pack": w8pack_dev, "wm32": wm_dev, "wv32": wv32_dev,
            "bcol": bcol_dev, "brow": brow_dev, "brow16": brow16_dev,
            "adjT": adjT_c.astype(ml_dtypes.bfloat16),
            "adjT8n": adjT8n_c,
        })
    return in_maps


LAST_RESULTS = None
LAST_NC = None


def kernel(**inputs):
    global LAST_RESULTS, LAST_NC
    from concourse.bass_utils import run_bass_kernel_spmd

    in_maps = host_prep(**inputs)
    nc = build_nc()
    LAST_NC = nc
    trace = bool(int(os.environ.get("KERNEL_TRACE", "0")))
    res = run_bass_kernel_spmd(nc, in_maps, core_ids=list(range(NCORES)),
                               trace=trace)
    LAST_RESULTS = res
    out = np.empty((B, D, N), np.float32)
    for c in range(NCORES):
        b, half = c // 2, c % 2
        out[b][:, half * NH:(half + 1) * NH] = res.results[c]["out"]
    return out



# revision 51
# speedup vs baseline: 1.1063x; 1.1063x over previous
"""Trainium2 Bass kernel for nn_MultiHeadedAttention_88021059764737.

Math (reference):
  q = Wq @ query + bq   (per batch; [D, N]), same k, v
  per head h (DIM=64): scores = q_h^T k_h / 8   [N, N]
  adj = dense adjacency counts from edges        [N, N]
  prob = exp(scores * adj) / rowsum
  x_h = v_h @ prob^T ; out = Wm @ x + bm

Device design (per core; 8 cores = 4 batches x 2 n-halves):
  - delta decomposition: exp(s*adj) = 1 + adj*(exp(s)-1); the "+1" part
    is restored analytically: vsum = Wv @ rowsum(value) + N*bv, injected
    into the PSUM accumulators via a K=1 matmul (rows 0..63 = vsum_h,
    row 64 = N for the denominator).
  - per (chunk, mtile): QK fp8 DoubleRow -> PSUM; then either
      * exp tile: ACT e = exp(s) (bf16), DVE ed = e*adj (2x tt), PE
        PV_e = vT @ ed (bf16) and PV_a = vT8 @ (-adj8) fp8 DR, or
      * poly tile (DVE-only, offloads the ACT bottleneck):
        ed = adj*(s + s^2/2) via ts(psum->bf16), ts, tt, tt; single
        PV_e matmul (no PV_a: the -adj is folded into the poly).
  - projections run as fp8e4m3 DoubleRow matmuls (host-quantized, x64
    scale folded out via the exp scale 2^-12); the [32p, 2-slot] DR
    layout comes from direct SBUF->SBUF fold DMAs (no DRAM bounce).
  - normalization: den row 64 of each accumulator -> DVE reciprocal ->
    gpsimd partition_broadcast -> DVE tt; out-projection in f32r with
    bm folded into the PSUM->SBUF staging copy.
"""

import os
import sys

sys.path.insert(0, "/opt/trn_rl_repo")

import numpy as np
import ml_dtypes

B, D, H, N, E = 4, 256, 4, 2048, 32768
DIM = D // H  # 64
NCORES = 8
NH = N // 2  # per-core n-half
SCALE = 1.0 / np.sqrt(np.float32(DIM))  # 1/8
ALPHA = 64.0  # fp8 weight scale; exp applies 1/ALPHA^2


def _poly_set():
    s = os.environ.get("KPOLY", "")
    out = set()
    if s.strip():
        for tok in s.split(","):
            c, mt = tok.split(":")
            out.add((int(c), int(mt)))
    return out


def build_nc(N_=N, NH_=NH):
    import concourse.bass as bass  # noqa: F401
    import concourse.mybir as mybir
    import concourse.tile as tile
    from concourse import bacc

    f32 = mybir.dt.float32
    f32r = mybir.dt.float32r
    bf16 = mybir.dt.bfloat16
    fp8 = mybir.dt.float8e4
    MT = N_ // 128          # m tiles of 128
    NCH = NH_ // 512        # n chunks of 512
    MG2 = MT // 2           # adj groups of 2 m-tiles
    Exp = mybir.ActivationFunctionType.Exp
    Ident = mybir.ActivationFunctionType.Identity
    mult = mybir.AluOpType.mult
    add = mybir.AluOpType.add
    DR = mybir.MatmulPerfMode.DoubleRow
    POLY = _poly_set()
    INV_A2 = 1.0 / (ALPHA * ALPHA)

    nc = bacc.Bacc()
    # ---- DRAM I/O ----
    xq8 = nc.dram_tensor("xq8", [128, 2, NH_], fp8, kind="ExternalInput")
    xk8 = nc.dram_tensor("xk8", [128, 2, N_], fp8, kind="ExternalInput")
    xv8 = nc.dram_tensor("xv8", [128, 2, N_], fp8, kind="ExternalInput")
    w8pack = nc.dram_tensor("w8pack", [128, 2, 3 * D], fp8,
                            kind="ExternalInput")  # [di, slot, wq|wk|wv]
    wm32 = nc.dram_tensor("wm32", [128, 2, D], f32r, kind="ExternalInput")
    bcol = nc.dram_tensor("bcol", [128, 6], f32,
                          kind="ExternalInput")  # bq8(2)|bk8(2)|bm(2)
    brow16 = nc.dram_tensor("brow16", [1, D], bf16,
                            kind="ExternalInput")  # a*bv in bf16
    adjT = nc.dram_tensor("adjT", [N_, NH_], bf16, kind="ExternalInput")
    adjT8n = nc.dram_tensor("adjT8n", [64, 2, MT, NH_], fp8,
                            kind="ExternalInput")  # -adj counts, m-folded
    # DRAM scratch for partition-fold bounces (SBUF partition dims cannot
    # be linearized into free-dim strides, so folds go via DRAM)
    qf8d = nc.dram_tensor("qf8d", [128, 2, NH_], fp8, kind="Internal")
    kf8d = nc.dram_tensor("kf8d", [128, 2, N_], fp8, kind="Internal")
    vt8d = nc.dram_tensor("vt8d", [128, MT, 4 * 65], fp8, kind="Internal")
    out = nc.dram_tensor("out", [D, NH_], f32, kind="ExternalOutput")

    from contextlib import ExitStack

    with tile.TileContext(nc) as tc, ExitStack() as ctx:
        consts = ctx.enter_context(tc.tile_pool(name="consts", bufs=1))
        big = ctx.enter_context(tc.tile_pool(name="big", bufs=1))
        work = ctx.enter_context(tc.tile_pool(name="work", bufs=2))
        pacc = ctx.enter_context(tc.tile_pool(name="pacc", bufs=1, space="PSUM"))
        psc = ctx.enter_context(tc.tile_pool(name="psc", bufs=2, space="PSUM"))

        # ---- warm the ACT exp table off the critical path ----
        warm = consts.tile([1, 2], f32, tag="warm")
        nc.vector.memset(warm[0:1, 0:1], 0.0)
        nc.scalar.activation(out=warm[0:1, 1:2], in_=warm[0:1, 0:1],
                             func=Exp, scale=1.0)

        # ---- constants; critical-path DMAs on SP first ----
        w8_sb = consts.tile([128, 2, 3 * D], fp8, tag="w8")
        wm_sb = consts.tile([128, 2, D], f32r, tag="wm")
        bcol_sb = consts.tile([128, 6], f32, tag="bcol")
        brow16_sb = consts.tile([1, D], bf16, tag="brow16")
        ones512_sb = consts.tile([1, 512], bf16, tag="ones512")
        ones16_sb = consts.tile([1, 128], bf16, tag="ones16")
        vsN_sb = consts.tile([1, 4, 65], bf16, tag="vsN")
        xk8_sb = big.tile([128, 2, N_], fp8, tag="xk8")
        xq8_sb = big.tile([128, 2, NH_], fp8, tag="xq8")
        xv8_sb = big.tile([128, 2, N_], fp8, tag="xv8")

        nc.sync.dma_start(out=w8_sb[:], in_=w8pack[:])
        nc.sync.dma_start(out=xq8_sb[:], in_=xq8[:])
        nc.sync.dma_start(out=bcol_sb[:], in_=bcol[:])
        nc.sync.dma_start(out=xk8_sb[:], in_=xk8[:])

        nc.vector.memset(ones512_sb[:], 1.0)
        nc.vector.memset(ones16_sb[:], 1.0)

        # non-critical consts on the ACT queue (idle early)
        nc.scalar.dma_start(out=brow16_sb[:], in_=brow16[:])
        nc.scalar.dma_start(out=wm_sb[:], in_=wm32[:])

        w8q = w8_sb[:, :, 0 * D:1 * D]
        w8k = w8_sb[:, :, 1 * D:2 * D]
        w8v = w8_sb[:, :, 2 * D:3 * D]

        # adjacency: 8 bf16 groups of 2 m-tiles; group 0 + adj8[0:4] early
        # on the Pool (SWDGE) queue to keep HWDGE free for the q/k chain.
        adj_g = [None] * MG2
        adj8_sb = big.tile([64, 2, MT, NH_], fp8, tag="adj8")

        def adj_tile(g, tag=None):
            adj_g[g] = big.tile([128, 2, NH_], bf16, tag=tag or f"adjE{g}",
                                name=f"adj{g}")

        def adj_load(g, eng, half=None):
            sl = slice(0, NH_) if half is None else \
                slice(512 * half, 512 * (half + 1))
            eng.dma_start(
                out=adj_g[g][:, :, sl],
                in_=adjT[256 * g:256 * (g + 1), sl].rearrange(
                    "(mt p) n -> p mt n", p=128),
            )

        def adj8_load(eng, mlo, mhi, half):
            sl = slice(512 * half, 512 * (half + 1))
            for s in range(2):
                eng.dma_start(out=adj8_sb[:, s, mlo:mhi, sl],
                              in_=adjT8n[:, s, mlo:mhi, sl])

        adj_tile(0)
        adj_load(0, nc.gpsimd, half=0)

        # ---- k/q projections (fp8 DR) -> SBUF->SBUF fold DMAs ----
        qf8_sb = big.tile([128, 2, NH_], fp8, tag="qf8")
        kf8_sb = big.tile([128, 2, N_], fp8, tag="kf8")
        k8_sb = big.tile([32, 2, 2, 2, N_], fp8, tag="k8")
        q8_sb = big.tile([32, 2, 2, 2, NH_], fp8, tag="q8")

        def fold_sbuf(fsb, fd, dst, mtile):
            # [h2*64+s*32+p, mtile, n] -> [p, h2, s, mtile, n] via DRAM
            nc.sync.dma_start(out=fd[:, mtile, :], in_=fsb[:, mtile, :])
            for h2 in range(2):
                nc.sync.dma_start(
                    out=dst[:, h2, :, mtile, :],
                    in_=fd[64 * h2:64 * h2 + 64, mtile, :].rearrange(
                        "(s p) n -> p s n", s=2, p=32))

        def emit_kproj(mtile):
            for np2 in range(N_ // 1024):
                ps = psc.tile([128, 1024], f32, tag="sc")
                for j in range(2):
                    ncol = np2 * 2 + j
                    nc.tensor.matmul(
                        ps[:, 512 * j:512 * (j + 1)],
                        lhsT=w8k[:, :, 128 * mtile:128 * (mtile + 1)],
                        rhs=xk8_sb[:, :, 512 * ncol:512 * (ncol + 1)],
                        start=True, stop=True, perf_mode=DR,
                    )
                if np2 == 0:
                    nc.scalar.activation(
                        out=kf8_sb[:, mtile, 1024 * np2:1024 * (np2 + 1)],
                        in_=ps[:], func=Ident,
                        bias=bcol_sb[:, 2 + mtile:3 + mtile], scale=1.0,
                    )
                else:
                    nc.vector.tensor_scalar(
                        out=kf8_sb[:, mtile, 1024 * np2:1024 * (np2 + 1)],
                        in0=ps[:], scalar1=bcol_sb[:, 2 + mtile:3 + mtile],
                        scalar2=None, op0=add,
                    )
            fold_sbuf(kf8_sb, kf8d, k8_sb, mtile)

        def emit_qproj(mtile):
            ps = psc.tile([128, 1024], f32, tag="sc")
            for ncol in range(NH_ // 512):
                nc.tensor.matmul(
                    ps[:, 512 * ncol:512 * (ncol + 1)],
                    lhsT=w8q[:, :, 128 * mtile:128 * (mtile + 1)],
                    rhs=xq8_sb[:, :, 512 * ncol:512 * (ncol + 1)],
                    start=True, stop=True, perf_mode=DR,
                )
            nc.vector.tensor_scalar(
                out=qf8_sb[:, mtile, :], in0=ps[:],
                scalar1=bcol_sb[:, mtile:mtile + 1], scalar2=None, op0=add,
            )
            fold_sbuf(qf8_sb, qf8d, q8_sb, mtile)

        emit_qproj(0)
        emit_kproj(0)
        emit_qproj(1)
        emit_kproj(1)

        nc.sync.dma_start(out=xv8_sb[:], in_=xv8[:])

        for g in range(1, MG2):
            adj_tile(g, tag=("xk8" if g == 5 else "xv8" if g == 6 else
                             "kf8" if g == 7 else None))

        # ---- v projection (fp8 DR + bf16 bias matmul) -> vT bf16 ----
        vT_sb = big.tile([128, MT, 4 * 65], bf16, tag="vT")
        vT8_sb = big.tile([64, 2, MT, 4 * 65], fp8, tag="vT8")
        nc.vector.memset(
            vT_sb.rearrange("q mt (h e) -> q mt h e", e=65)[:, :, :, 64:65],
            1.0)

        def emit_vproj_group(mtg):
            ps = psc.tile([128, 1024], f32, tag="sc")
            for j in range(4):
                mt = mtg * 4 + j
                nc.tensor.matmul(
                    ps[:, 256 * j:256 * (j + 1)],
                    lhsT=xv8_sb[:, :, 128 * mt:128 * (mt + 1)],
                    rhs=w8v[:], start=True, stop=False, perf_mode=DR,
                )
                nc.tensor.matmul(  # + ALPHA*bv via K=1 ones row (bf16 rate)
                    ps[:, 256 * j:256 * (j + 1)],
                    lhsT=ones16_sb[0:1, 0:128], rhs=brow16_sb[0:1, 0:D],
                    start=False, stop=True,
                )
            vout = vT_sb.rearrange("q mt (h e) -> q mt h e", e=65)[
                :, 4 * mtg:4 * (mtg + 1), :, 0:64]
            vin = ps[:].rearrange("q (mtl h d) -> q mtl h d", h=4, d=64)
            nc.scalar.activation(out=vout, in_=vin, func=Ident,
                                 scale=1.0 / ALPHA)
            # fold to fp8 [64, 2slot, mt, 260] via DRAM (casting gpsimd leg)
            msl = slice(4 * mtg, 4 * (mtg + 1))
            nc.gpsimd.dma_start(out=vt8d[:, msl, :], in_=vT_sb[:, msl, :])
            nc.gpsimd.dma_start(
                out=vT8_sb[:, :, msl, :],
                in_=vt8d[:, msl, :].rearrange("(s p) mt e -> p s mt e",
                                              s=2, p=64))

        # ---- vsum via vT's ones column: vsN[h, :] = [sum_m vT_h; N] ----
        # row-form directly: out[0, 65h+i] = sum_m ones[m] * vT[m, 65h+i]
        # = (Wv@value + N*bv)[h*64+i] exactly (bias is in vT); the ones
        # column (i=64) sums to N.
        ones_col = consts.tile([128, 1], bf16, tag="onescol")
        nc.vector.memset(ones_col[:], 1.0)

        for mtg in range(4):
            emit_vproj_group(mtg)
        adj8_load(nc.gpsimd, 0, 4, 0)

        # vsum in the prologue: PE is gated on the q/k folds here anyway,
        # and the pacc x0 bank is free until attention starts.
        vs_ps = pacc.tile([128, 512], f32, tag="x0", name="vs_ps")
        for mt in range(MT):
            nc.tensor.matmul(
                vs_ps[0:1, 0:4 * 65],
                lhsT=ones_col[:],
                rhs=vT_sb[:, mt, :],
                start=(mt == 0), stop=(mt == MT - 1),
            )
        nc.vector.tensor_copy(
            out=vsN_sb[:],
            in_=vs_ps[0:1, 0:260].rearrange("q (h e) -> q h e", h=4))

        # remaining bulk loads on the now-idle SP queue (HWDGE)
        for g in range(1, MG2):
            adj_load(g, nc.sync, half=0)
        adj8_load(nc.sync, 4, MT, 0)
        adj_load(0, nc.sync, half=1)
        for g in range(1, MG2):
            adj_load(g, nc.sync, half=1)
        adj8_load(nc.sync, 0, 8, 1)
        adj8_load(nc.sync, 8, MT, 1)

        # ---- attention ----
        # PE issue order is software-pipelined: QK(t+1) is emitted BEFORE
        # PV(t) so the in-order PE stream never stalls behind the
        # ACT/DVE chain of the current tile.
        x_h_of = {}
        xsb_of = {}
        scp_of = {}

        def emit_qk(c, mt):
            nsl = slice(512 * c, 512 * (c + 1))
            tiles = []
            for p in range(2):
                scp = psc.tile([128, 1024], f32, tag="sc", name="scp")
                tiles.append(scp)
                for h2 in range(2):
                    h = 2 * p + h2
                    nc.tensor.matmul(
                        scp[:, 512 * h2:512 * (h2 + 1)],
                        lhsT=k8_sb[:, h % 2, :, h // 2,
                                   128 * mt:128 * (mt + 1)],
                        rhs=q8_sb[:, h % 2, :, h // 2, nsl],
                        start=True, stop=True, perf_mode=DR,
                    )
            scp_of[(c, mt)] = tiles

        def emit_ew(c, mt):
            nsl = slice(512 * c, 512 * (c + 1))
            poly = (c, mt) in POLY
            adjtile = adj_g[mt // 2]
            adj_sl = adjtile[:, mt % 2, nsl]
            adj_b = bass.AP(tensor=adj_sl.tensor, offset=adj_sl.offset,
                            ap=[adj_sl.ap[0], [0, 4], adj_sl.ap[-1]])
            scps = scp_of.pop((c, mt))
            if poly:
                s16 = work.tile([128, 4, 512], bf16, tag="exp", name="s16",
                                bufs=4)
                pp = work.tile([128, 4, 512], bf16, tag="pp", name="pp",
                               bufs=2)
                pp2 = work.tile([128, 4, 512], bf16, tag="pp", name="pp2",
                                bufs=2)
                for p in range(2):
                    nc.vector.tensor_scalar(
                        out=s16[:, 2 * p:2 * p + 2, :], in0=scps[p][:],
                        scalar1=INV_A2, scalar2=None, op0=mult,
                    )
            else:
                e_t = work.tile([128, 4, 512], bf16, tag="exp", name="e_t",
                                bufs=4)
                for p in range(2):
                    nc.scalar.activation(
                        out=e_t[:, 2 * p:2 * p + 2, :], in_=scps[p][:],
                        func=Exp, scale=INV_A2,
                    )
            ed_t = work.tile([128, 4, 512], bf16, tag="dlt", name="ed_t",
                             bufs=4)
            if poly:
                # ed = adj * (s + s^2/2)  [= adj*(e-1), PV_a folded in]
                nc.vector.tensor_scalar(out=pp[:], in0=s16[:], scalar1=0.5,
                                        scalar2=1.0, op0=mult, op1=add)
                nc.vector.tensor_tensor(out=pp2[:], in0=pp[:], in1=s16[:],
                                        op=mult)
                nc.vector.tensor_tensor(out=ed_t[:], in0=pp2[:], in1=adj_b,
                                        op=mult)
            else:
                nc.vector.tensor_tensor(out=ed_t[:], in0=e_t[:], in1=adj_b,
                                        op=mult)
            return ed_t

        def emit_pv(c, mt, ed_t):
            nsl = slice(512 * c, 512 * (c + 1))
            x_h = x_h_of[c]
            poly = (c, mt) in POLY
            last = mt == MT - 1
            assert not (poly and last)
            for hh in range(4):
                nc.tensor.matmul(
                    x_h[hh][0:65, :],
                    lhsT=vT_sb[:, mt, 65 * hh:65 * (hh + 1)],
                    rhs=ed_t[:, hh, :],
                    start=(mt == 0), stop=False,
                )
                if not poly:
                    nc.tensor.matmul(  # -= vT @ adj  (fp8 DoubleRow)
                        x_h[hh][0:65, :],
                        lhsT=vT8_sb[:, :, mt, 65 * hh:65 * (hh + 1)],
                        rhs=adj8_sb[:, :, mt, nsl],
                        start=False, stop=last, perf_mode=DR,
                    )

        def emit_accfin(c):
            # += [vsum_h; N] broadcast along n (plain accumulation)
            x_h = x_h_of[c]
            for hh in range(4):
                nc.tensor.matmul(
                    x_h[hh][0:65, :],
                    lhsT=vsN_sb[0:1, hh, :],
                    rhs=ones512_sb[0:1, :],
                    start=False, stop=False,
                )

        norm_tiles = {}

        def emit_norm(c, heads=range(4)):
            # xsb = x * (1 / den); den = row 64 (already includes +N)
            x_h = x_h_of[c]
            if c not in norm_tiles:
                norm_tiles[c] = (
                    work.tile([1, 4, 512], f32, tag="den", name="den",
                              bufs=1),
                    work.tile([128, 4, 512], f32, tag="denb", name="den_b",
                              bufs=1),
                    work.tile([128, 2, 512], f32r, tag="xsb", name="xsb"),
                )
            denom_t, den_b, xsb = norm_tiles[c]
            xsb_of[c] = xsb
            for hh in heads:
                nc.vector.reciprocal(out=denom_t[0:1, hh, :],
                                     in_=x_h[hh][64:65, :])
                nc.gpsimd.partition_broadcast(
                    den_b[:, hh, :], denom_t[0:1, hh, :])
            for hh in heads:
                h2, kc = hh % 2, hh // 2
                nc.vector.tensor_tensor(
                    out=xsb[64 * h2:64 * h2 + 64, kc, :],
                    in0=x_h[hh][0:64, :],
                    in1=den_b[64 * h2:64 * h2 + 64, hh, :],
                    op=mult,
                )

        def emit_outproj(c, on_act=False):
            nsl = slice(512 * c, 512 * (c + 1))
            xsb = xsb_of[c]
            op_ps = psc.tile([128, 1024], f32, tag="sc", name="op_ps")
            out_t = work.tile([128, 1024], f32, tag="osb", name="out_t")
            for kc in range(2):
                for mtile in range(2):
                    nc.tensor.matmul(
                        op_ps[:, 512 * mtile:512 * (mtile + 1)],
                        lhsT=wm_sb[:, kc, 128 * mtile:128 * (mtile + 1)],
                        rhs=xsb[:, kc, :],
                        start=(kc == 0), stop=(kc == 1),
                    )
            for mtile in range(2):
                if on_act:  # tail: ACT is idle, DVE is the critical chain
                    nc.scalar.activation(
                        out=out_t[:, 512 * mtile:512 * (mtile + 1)],
                        in_=op_ps[:, 512 * mtile:512 * (mtile + 1)],
                        func=Ident, bias=bcol_sb[:, 4 + mtile:5 + mtile],
                        scale=1.0,
                    )
                else:
                    nc.vector.tensor_scalar(
                        out=out_t[:, 512 * mtile:512 * (mtile + 1)],
                        in0=op_ps[:, 512 * mtile:512 * (mtile + 1)],
                        scalar1=bcol_sb[:, 4 + mtile:5 + mtile],
                        scalar2=None, op0=add,
                    )
                nc.sync.dma_start(
                    out=out[128 * mtile:128 * (mtile + 1), nsl],
                    in_=out_t[:, 512 * mtile:512 * (mtile + 1)])

        for c in range(NCH):
            x_h_of[c] = [pacc.tile([128, 512], f32, tag=f"x{hh}",
                                   name=f"x{hh}") for hh in range(4)]
        tiles = [(c, mt) for c in range(NCH) for mt in range(MT)]
        emit_qk(*tiles[0])
        emit_qk(*tiles[1])
        for i, (c, mt) in enumerate(tiles):
            if i + 2 < len(tiles):
                emit_qk(*tiles[i + 2])
            ed_t = emit_ew(c, mt)
            emit_pv(c, mt, ed_t)
            if mt == 8:
                emit_accfin(c)
            if c > 0 and mt in (2, 5, 8, 11):
                emit_norm(c - 1, heads=[(mt - 2) // 3])  # spread per head
            if c > 0 and mt == 13:
                emit_outproj(c - 1)    # previous chunk's projection
        # keep the PE clock warm through the final norm window so the
        # out-projection runs at full p-state (dummy QK-shaped matmuls)
        warm_ps = psc.tile([128, 1024], f32, tag="sc", name="warm_ps")
        for j in range(8):
            nc.tensor.matmul(
                warm_ps[:, 512 * (j % 2):512 * (j % 2 + 1)],
                lhsT=k8_sb[:, 0, :, 0, 0:128],
                rhs=q8_sb[:, 0, :, 0, 0:512],
                start=True, stop=True, perf_mode=DR,
            )
        emit_norm(NCH - 1)
        emit_outproj(NCH - 1, on_act=True)

    nc.compile()
    return nc


def host_prep(query, key, value, edges, Wq, bq, Wk, bk, Wv, bv, Wm, bm,
              N_=N, NH_=NH, B_=B):
    """Returns per-core input maps."""
    f32 = np.float32
    fp8 = ml_dtypes.float8_e4m3
    query = np.asarray(query, f32)
    key = np.asarray(key, f32)
    value = np.asarray(value, f32)
    edges = np.asarray(edges)
    Wq, bq = np.asarray(Wq, f32), np.asarray(bq, f32)
    Wk, bk = np.asarray(Wk, f32), np.asarray(bk, f32)
    Wv, bv = np.asarray(Wv, f32), np.asarray(bv, f32)
    Wm, bm = np.asarray(Wm, f32), np.asarray(bm, f32)
    MT = N_ // 128

    # head-major permutation: dd = h*DIM + dl  <->  o = dl*H + h
    dd = np.arange(D)
    perm = (dd % DIM) * H + (dd // DIM)

    def w8_layout(WT):  # WT [256(K=d_in), 256(dd)] -> [128, 2, 256] fp8
        return np.ascontiguousarray(
            WT.reshape(2, 128, D).transpose(1, 0, 2)).astype(fp8)

    def w32_layout(WT):
        return np.ascontiguousarray(WT.reshape(2, 128, D).transpose(1, 0, 2))

    wq8 = w8_layout((Wq[perm, :] * (ALPHA * SCALE)).T)
    wk8 = w8_layout((Wk[perm, :] * ALPHA).T)
    wv8 = w8_layout((Wv[perm, :] * ALPHA).T)
    w8pack_dev = np.ascontiguousarray(np.concatenate([wq8, wk8, wv8], axis=2))
    wm_dev = w32_layout(Wm[:, perm].T)
    bq8 = np.ascontiguousarray((bq[perm] * (ALPHA * SCALE)).reshape(2, 128).T)
    bk8 = np.ascontiguousarray((bk[perm] * ALPHA).reshape(2, 128).T)
    bm_col = np.ascontiguousarray(bm.reshape(2, 128).T)
    bcol_dev = np.ascontiguousarray(
        np.concatenate([bq8, bk8, bm_col], axis=1))
    brow16_dev = np.ascontiguousarray(
        (bv[perm] * ALPHA).reshape(1, D)).astype(ml_dtypes.bfloat16)

    def fold_x(x):  # [256, n] -> [128, 2, n] fp8
        return np.ascontiguousarray(
            x.reshape(2, 128, x.shape[1]).transpose(1, 0, 2)).astype(fp8)

    in_maps = []
    ncores = 2 * B_
    for c in range(ncores):
        b, half = c // 2, c % 2
        ns = slice(half * NH_, (half + 1) * NH_)
        adj = np.zeros((N_, N_), f32)
        np.add.at(adj, (edges[b, 0].astype(np.int64),
                        edges[b, 1].astype(np.int64)), 1.0)
        adjT_c = np.ascontiguousarray(adj[ns, :].T)
        # fp8 negated, m-folded: [64, 2slot, mt, n]
        adjT8n_c = np.ascontiguousarray(
            (-adjT_c).reshape(MT, 2, 64, NH_).transpose(2, 1, 0, 3)
        ).astype(fp8)
        in_maps.append({
            "xq8": fold_x(query[b][:, ns]),
            "xk8": fold_x(key[b]),
            "xv8": fold_x(value[b]),
            "w8pack": w8pack_dev, "wm32": wm_dev,
            "bcol": bcol_dev, "brow16": brow16_dev,
            "adjT": adjT_c.astype(ml_dtypes.bfloat16),
            "adjT8n": adjT8n_c,
        })
    return in_maps


LAST_RESULTS = None
LAST_NC = None


def kernel(**inputs):
    global LAST_RESULTS, LAST_NC
    from concourse.bass_utils import run_bass_kernel_spmd

    in_maps = host_prep(**inputs)
    nc = build_nc()
    LAST_NC = nc
    trace = bool(int(os.environ.get("KERNEL_TRACE", "0")))
    res = run_bass_kernel_spmd(nc, in_maps, core_ids=list(range(NCORES)),
                               trace=trace)
    LAST_RESULTS = res
    out = np.empty((B, D, N), np.float32)
    for c in range(NCORES):
        b, half = c // 2, c % 2
        out[b][:, half * NH:(half + 1) * NH] = res.results[c]["out"]
    return out


# revision 89
# speedup vs baseline: 1.1880x; 1.0738x over previous
"""Trainium2 Bass kernel for nn_MultiHeadedAttention_88021059764737.

Math (reference):
  q = Wq @ query + bq   (per batch; [D, N]), same k, v
  per head h (DIM=64): scores = q_h^T k_h / 8   [N, N]
  adj = dense adjacency counts from edges        [N, N]
  prob = exp(scores * adj) / rowsum
  x_h = v_h @ prob^T ; out = Wm @ x + bm

Device design (per core; 8 cores = 4 batches x 2 n-halves):
  - delta decomposition: exp(s*adj) = 1 + adj*(exp(s)-1); the "+1" part
    is restored analytically: vsum[h] = sum_m vT_h (exactly
    Wv@rowsum(value) + N*bv since the bias is in vT; the ones column
    gives +N for the denominator), injected into each PSUM accumulator
    via a K=1 matmul.
  - per (chunk, mtile): QK fp8 DoubleRow -> PSUM, ACT e = exp(s)
    (bf16), DVE ed = e*adj (2x tt), PE PV_e = vT @ ed (bf16) and
    PV_a = vT8 @ (-adj8) fp8 DR into the same banks.
  - the PE issue stream is software-pipelined (QK of tile t+2 emitted
    before PV of tile t) so the in-order PE never stalls the
    QK->exp->mult->PV ring; the first tiles use non-DoubleRow QK
    straight off the unfolded kf8/qf8 so the fold DMAs leave the
    critical path; the first tiles' PV_a are deferred until vT8/adj8
    land (accumulation order is free).
  - projections run as fp8e4m3 DoubleRow matmuls (host-quantized, x64
    scale folded out via the exp scale 2^-12); DR layouts fold via
    DRAM bounce DMAs (SBUF partition dims can't be restrided).
  - normalization: den row 64 of each accumulator -> DVE reciprocal ->
    gpsimd partition_broadcast -> DVE tt, spread one head per m-tile
    slot; out-projection in f32r with bm folded into the staging copy;
    dummy warm matmuls keep the PE p-state up through the tail.
"""

import os
import sys

sys.path.insert(0, "/opt/trn_rl_repo")

import numpy as np
import ml_dtypes

B, D, H, N, E = 4, 256, 4, 2048, 32768
DIM = D // H  # 64
NCORES = 8
NH = N // 2  # per-core n-half
SCALE = 1.0 / np.sqrt(np.float32(DIM))  # 1/8
ALPHA = 64.0  # fp8 weight scale; exp applies 1/ALPHA^2


def _poly_set():
    s = os.environ.get("KPOLY", "")
    out = set()
    if s.strip():
        for tok in s.split(","):
            c, mt = tok.split(":")
            out.add((int(c), int(mt)))
    return out


def build_nc(N_=N, NH_=NH):
    import concourse.bass as bass  # noqa: F401
    import concourse.mybir as mybir
    import concourse.tile as tile
    from concourse import bacc

    f32 = mybir.dt.float32
    f32r = mybir.dt.float32r
    bf16 = mybir.dt.bfloat16
    fp8 = mybir.dt.float8e4
    MT = N_ // 128          # m tiles of 128
    NCH = NH_ // 512        # n chunks of 512
    MG2 = MT // 2           # adj groups of 2 m-tiles
    Exp = mybir.ActivationFunctionType.Exp
    Ident = mybir.ActivationFunctionType.Identity
    mult = mybir.AluOpType.mult
    add = mybir.AluOpType.add
    DR = mybir.MatmulPerfMode.DoubleRow
    POLY = _poly_set()
    INV_A2 = 1.0 / (ALPHA * ALPHA)

    nc = bacc.Bacc()
    # ---- DRAM I/O ----
    xq8 = nc.dram_tensor("xq8", [128, 2, NH_], fp8, kind="ExternalInput")
    xk8 = nc.dram_tensor("xk8", [128, 2, N_], fp8, kind="ExternalInput")
    xv8 = nc.dram_tensor("xv8", [128, 2, N_], fp8, kind="ExternalInput")
    w8pack = nc.dram_tensor("w8pack", [128, 2, 3 * D], fp8,
                            kind="ExternalInput")  # [di, slot, wq|wk|wv]
    wm32 = nc.dram_tensor("wm32", [128, 2, D], f32r, kind="ExternalInput")
    bcol = nc.dram_tensor("bcol", [128, 6], f32,
                          kind="ExternalInput")  # bq8(2)|bk8(2)|bm(2)
    brow16 = nc.dram_tensor("brow16", [1, D], bf16,
                            kind="ExternalInput")  # a*bv in bf16
    adjT = nc.dram_tensor("adjT", [N_, NH_], bf16, kind="ExternalInput")
    adjT8n = nc.dram_tensor("adjT8n", [64, 2, MT, NH_], fp8,
                            kind="ExternalInput")  # -adj counts, m-folded
    # DRAM scratch for partition-fold bounces (SBUF partition dims cannot
    # be linearized into free-dim strides, so folds go via DRAM)
    qf8d = nc.dram_tensor("qf8d", [128, 2, NH_], fp8, kind="Internal")
    kf8d = nc.dram_tensor("kf8d", [128, 2, N_], fp8, kind="Internal")
    vt8d = nc.dram_tensor("vt8d", [128, MT, 4 * 65], fp8, kind="Internal")
    out = nc.dram_tensor("out", [D, NH_], f32, kind="ExternalOutput")

    from contextlib import ExitStack

    with tile.TileContext(nc) as tc, ExitStack() as ctx:
        consts = ctx.enter_context(tc.tile_pool(name="consts", bufs=1))
        big = ctx.enter_context(tc.tile_pool(name="big", bufs=1))
        work = ctx.enter_context(tc.tile_pool(name="work", bufs=2))
        pacc = ctx.enter_context(tc.tile_pool(name="pacc", bufs=1, space="PSUM"))
        psc = ctx.enter_context(tc.tile_pool(name="psc", bufs=2, space="PSUM"))

        # ---- warm the ACT exp table off the critical path ----
        warm = consts.tile([1, 2], f32, tag="warm")
        nc.vector.memset(warm[0:1, 0:1], 0.0)
        nc.scalar.activation(out=warm[0:1, 1:2], in_=warm[0:1, 0:1],
                             func=Exp, scale=1.0)

        # ---- constants; critical-path DMAs on SP first ----
        w8_sb = consts.tile([128, 2, 3 * D], fp8, tag="w8")
        wm_sb = consts.tile([128, 2, D], f32r, tag="wm")
        bcol_sb = consts.tile([128, 6], f32, tag="bcol")
        brow16_sb = consts.tile([1, D], bf16, tag="brow16")
        ones512_sb = consts.tile([1, 512], bf16, tag="ones512")
        ones16_sb = consts.tile([1, 128], bf16, tag="ones16")
        vsN_sb = consts.tile([1, 4, 65], bf16, tag="vsN")
        xk8_sb = big.tile([128, 2, N_], fp8, tag="xk8")
        xq8_sb = big.tile([128, 2, NH_], fp8, tag="xq8")
        xv8_sb = big.tile([128, 2, N_], fp8, tag="xv8")

        nc.sync.dma_start(out=w8_sb[:], in_=w8pack[:])
        nc.sync.dma_start(out=xk8_sb[:], in_=xk8[:])
        nc.sync.dma_start(out=bcol_sb[:], in_=bcol[:])
        nc.sync.dma_start(out=xq8_sb[:], in_=xq8[:])

        nc.vector.memset(ones512_sb[:], 1.0)
        nc.vector.memset(ones16_sb[:], 1.0)



        w8q = w8_sb[:, :, 0 * D:1 * D]
        w8k = w8_sb[:, :, 1 * D:2 * D]
        w8v = w8_sb[:, :, 2 * D:3 * D]

        # adjacency: 8 bf16 groups of 2 m-tiles; group 0 + adj8[0:4] early
        # on the Pool (SWDGE) queue to keep HWDGE free for the q/k chain.
        adj_g = [None] * MG2
        adj8_sb = big.tile([64, 2, MT, NH_], fp8, tag="adj8")

        def adj_tile(g, tag=None):
            adj_g[g] = big.tile([128, 2, NH_], bf16, tag=tag or f"adjE{g}",
                                name=f"adj{g}")

        def adj_load(g, eng, half=None):
            sl = slice(0, NH_) if half is None else \
                slice(512 * half, 512 * (half + 1))
            eng.dma_start(
                out=adj_g[g][:, :, sl],
                in_=adjT[256 * g:256 * (g + 1), sl].rearrange(
                    "(mt p) n -> p mt n", p=128),
            )

        def adj8_load(eng, mlo, mhi, half):
            sl = slice(512 * half, 512 * (half + 1))
            for s in range(2):
                eng.dma_start(out=adj8_sb[:, s, mlo:mhi, sl],
                              in_=adjT8n[:, s, mlo:mhi, sl])

        adj_tile(0)
        adj_load(0, nc.gpsimd, half=0)

        # ---- k/q projections (fp8 DR) -> SBUF->SBUF fold DMAs ----
        qf8_sb = big.tile([128, 2, NH_], fp8, tag="qf8")
        kf8_sb = big.tile([128, 2, N_], fp8, tag="kf8")
        k8_sb = big.tile([32, 2, 2, 2, N_], fp8, tag="k8")
        q8_sb = big.tile([32, 2, 2, 2, NH_], fp8, tag="q8")

        def fold_store(fsb, fd, mtile, sl):
            nc.sync.dma_start(out=fd[:, mtile, sl], in_=fsb[:, mtile, sl])

        def fold_load(fd, dst, mtile, sl):
            # [h2*64+s*32+p, mtile, n] -> [p, h2, s, mtile, n] via DRAM;
            # split by n-chunk so the first QK waits only for chunk 0
            for h2 in range(2):
                nc.sync.dma_start(
                    out=dst[:, h2, :, mtile, sl],
                    in_=fd[64 * h2:64 * h2 + 64, mtile, sl].rearrange(
                        "(s p) n -> p s n", s=2, p=32))

        def emit_kproj(mtile):
            for np2 in range(N_ // 1024):
                ps = psc.tile([128, 1024], f32, tag="sc")
                for j in range(2):
                    ncol = np2 * 2 + j
                    nc.tensor.matmul(
                        ps[:, 512 * j:512 * (j + 1)],
                        lhsT=w8k[:, :, 128 * mtile:128 * (mtile + 1)],
                        rhs=xk8_sb[:, :, 512 * ncol:512 * (ncol + 1)],
                        start=True, stop=True, perf_mode=DR,
                    )
                if np2 == 0:
                    nc.scalar.activation(
                        out=kf8_sb[:, mtile, 1024 * np2:1024 * (np2 + 1)],
                        in_=ps[:], func=Ident,
                        bias=bcol_sb[:, 2 + mtile:3 + mtile], scale=1.0,
                    )
                else:
                    nc.vector.tensor_scalar(
                        out=kf8_sb[:, mtile, 1024 * np2:1024 * (np2 + 1)],
                        in0=ps[:], scalar1=bcol_sb[:, 2 + mtile:3 + mtile],
                        scalar2=None, op0=add,
                    )
                fold_store(kf8_sb, kf8d, mtile, slice(1024 * np2,
                                                      1024 * (np2 + 1)))
            # fold loads deferred to the bulk section (nodr covers t0..t3)

        def emit_qproj(mtile):
            ps = psc.tile([128, 1024], f32, tag="sc")
            for ncol in range(NH_ // 512):
                nc.tensor.matmul(
                    ps[:, 512 * ncol:512 * (ncol + 1)],
                    lhsT=w8q[:, :, 128 * mtile:128 * (mtile + 1)],
                    rhs=xq8_sb[:, :, 512 * ncol:512 * (ncol + 1)],
                    start=True, stop=True, perf_mode=DR,
                )
            nc.vector.tensor_scalar(
                out=qf8_sb[:, mtile, :], in0=ps[:],
                scalar1=bcol_sb[:, mtile:mtile + 1], scalar2=None, op0=add,
            )
            fold_store(qf8_sb, qf8d, mtile, slice(0, NH_))



        for g in range(1, MG2):
            adj_tile(g, tag=("xk8" if g == 5 else "xv8" if g == 6 else
                             "kf8" if g == 7 else None))

        # ---- v projection (fp8 DR + bf16 bias matmul) -> vT bf16 ----
        vT_sb = big.tile([128, MT, 4 * 65], bf16, tag="vT")
        vT8_sb = big.tile([64, 2, MT, 4 * 65], fp8, tag="vT8")
        nc.vector.memset(
            vT_sb.rearrange("q mt (h e) -> q mt h e", e=65)[:, :, :, 64:65],
            1.0)

        def emit_vproj_group(mtg):
            ps = psc.tile([128, 1024], f32, tag="sc")
            for j in range(4):
                mt = mtg * 4 + j
                nc.tensor.matmul(
                    ps[:, 256 * j:256 * (j + 1)],
                    lhsT=xv8_sb[:, :, 128 * mt:128 * (mt + 1)],
                    rhs=w8v[:], start=True, stop=False, perf_mode=DR,
                )
                nc.tensor.matmul(  # + ALPHA*bv via K=1 ones row (bf16 rate)
                    ps[:, 256 * j:256 * (j + 1)],
                    lhsT=ones16_sb[0:1, 0:128], rhs=brow16_sb[0:1, 0:D],
                    start=False, stop=True,
                )
            vout = vT_sb.rearrange("q mt (h e) -> q mt h e", e=65)[
                :, 4 * mtg:4 * (mtg + 1), :, 0:64]
            vin = ps[:].rearrange("q (mtl h d) -> q mtl h d", h=4, d=64)
            nc.scalar.activation(out=vout, in_=vin, func=Ident,
                                 scale=1.0 / ALPHA)
            # fold to fp8 [64, 2slot, mt, 260] via DRAM (casting gpsimd
            # store leg; plain fp8 load leg rides HWDGE)
            msl = slice(4 * mtg, 4 * (mtg + 1))
            nc.gpsimd.dma_start(out=vt8d[:, msl, :], in_=vT_sb[:, msl, :])
            nc.gpsimd.dma_start(
                out=vT8_sb[:, :, msl, :],
                in_=vt8d[:, msl, :].rearrange("(s p) mt e -> p s mt e",
                                              s=2, p=64))

        # ---- vsum via vT's ones column: vsN[h, :] = [sum_m vT_h; N] ----
        # row-form directly: out[0, 65h+i] = sum_m ones[m] * vT[m, 65h+i]
        # = (Wv@value + N*bv)[h*64+i] exactly (bias is in vT); the ones
        # column (i=64) sums to N.
        ones_col = consts.tile([128, 1], bf16, tag="onescol")
        nc.vector.memset(ones_col[:], 1.0)

        emit_kproj(0)
        emit_qproj(0)
        nc.sync.dma_start(out=brow16_sb[:], in_=brow16[:])
        emit_qproj(1)
        emit_kproj(1)
        for mtl in range(2):
            nc.sync.dma_start(
                out=adj_g[1][:, mtl, 0:512],
                in_=adjT[256 + 128 * mtl:256 + 128 * (mtl + 1), 0:512]
                .rearrange("(mt p) n -> p mt n", p=128))
        adj_load(2, nc.sync, half=0)
        nc.sync.dma_start(out=xv8_sb[:], in_=xv8[:])
        adj_load(3, nc.sync, half=0)

        def emit_vproj_all():
            for mtg in range(4):
                emit_vproj_group(mtg)
            adj8_load(nc.gpsimd, 0, 4, 0)

        # vsum accumulator bank reserved now; matmuls emitted after the
        # first QKs so the burst doesn't block them on the in-order PE.
        vs_ps = pacc.tile([128, 512], f32, tag="x0", name="vs_ps")

        def emit_vsum():
            for mt in range(MT):
                nc.tensor.matmul(
                    vs_ps[0:1, 0:4 * 65],
                    lhsT=ones_col[:],
                    rhs=vT_sb[:, mt, :],
                    start=(mt == 0), stop=(mt == MT - 1),
                )
            nc.vector.tensor_copy(
                out=vsN_sb[:],
                in_=vs_ps[0:1, 0:260].rearrange("q (h e) -> q h e", h=4))

        # remaining bulk loads on the now-idle SP queue (HWDGE), ordered
        # by deadline; deferred fold chunks interleave by need
        fold_load(kf8d, k8_sb, 0, slice(0, 1024))
        fold_load(kf8d, k8_sb, 1, slice(0, 1024))
        fold_load(qf8d, q8_sb, 0, slice(0, 512))
        fold_load(qf8d, q8_sb, 1, slice(0, 512))
        fold_load(kf8d, k8_sb, 0, slice(1024, 2048))
        fold_load(kf8d, k8_sb, 1, slice(1024, 2048))
        for g in range(4, MG2):
            adj_load(g, nc.sync, half=0)
        adj8_load(nc.sync, 4, MT, 0)
        nc.sync.dma_start(out=wm_sb[:], in_=wm32[:])
        fold_load(qf8d, q8_sb, 0, slice(512, 1024))
        fold_load(qf8d, q8_sb, 1, slice(512, 1024))
        adj_load(0, nc.sync, half=1)
        for g in range(1, MG2):
            adj_load(g, nc.sync, half=1)
        adj8_load(nc.sync, 0, 8, 1)
        adj8_load(nc.sync, 8, MT, 1)

        # ---- attention ----
        # PE issue order is software-pipelined: QK(t+1) is emitted BEFORE
        # PV(t) so the in-order PE stream never stalls behind the
        # ACT/DVE chain of the current tile.
        x_h_of = {}
        xsb_of = {}
        scp_of = {}

        def emit_qk(c, mt, nodr=False):
            nsl = slice(512 * c, 512 * (c + 1))
            tiles = []
            for p in range(2):
                scp = psc.tile([128, 1024], f32, tag="sc", name="scp")
                tiles.append(scp)
                for h2 in range(2):
                    h = 2 * p + h2
                    if nodr:
                        # unfolded head-major rows; skips the fold deadline
                        rsl = slice(64 * (h % 2), 64 * (h % 2) + 64)
                        nc.tensor.matmul(
                            scp[:, 512 * h2:512 * (h2 + 1)],
                            lhsT=kf8_sb[rsl, h // 2,
                                        128 * mt:128 * (mt + 1)],
                            rhs=qf8_sb[rsl, h // 2, nsl],
                            start=True, stop=True,
                        )
                    else:
                        nc.tensor.matmul(
                            scp[:, 512 * h2:512 * (h2 + 1)],
                            lhsT=k8_sb[:, h % 2, :, h // 2,
                                       128 * mt:128 * (mt + 1)],
                            rhs=q8_sb[:, h % 2, :, h // 2, nsl],
                            start=True, stop=True, perf_mode=DR,
                        )
            scp_of[(c, mt)] = tiles

        def emit_ew(c, mt):
            nsl = slice(512 * c, 512 * (c + 1))
            poly = (c, mt) in POLY
            adjtile = adj_g[mt // 2]
            adj_sl = adjtile[:, mt % 2, nsl]
            adj_b = bass.AP(tensor=adj_sl.tensor, offset=adj_sl.offset,
                            ap=[adj_sl.ap[0], [0, 4], adj_sl.ap[-1]])
            scps = scp_of.pop((c, mt))
            if poly:
                s16 = work.tile([128, 4, 512], bf16, tag="exp", name="s16",
                                bufs=4)
                pp = work.tile([128, 4, 512], bf16, tag="pp", name="pp",
                               bufs=2)
                pp2 = work.tile([128, 4, 512], bf16, tag="pp", name="pp2",
                                bufs=2)
                for p in range(2):
                    nc.vector.tensor_scalar(
                        out=s16[:, 2 * p:2 * p + 2, :], in0=scps[p][:],
                        scalar1=INV_A2, scalar2=None, op0=mult,
                    )
            else:
                e_t = work.tile([128, 4, 512], bf16, tag="exp", name="e_t",
                                bufs=5)
                for p in range(2):
                    nc.scalar.activation(
                        out=e_t[:, 2 * p:2 * p + 2, :], in_=scps[p][:],
                        func=Exp, scale=INV_A2,
                    )
            ed_t = work.tile([128, 4, 512], bf16, tag="dlt", name="ed_t",
                             bufs=5)
            last = c == NCH - 1 and mt == MT - 1
            if last:
                for p in range(2):
                    nc.vector.tensor_tensor(
                        out=ed_t[:, 2 * p:2 * p + 2, :],
                        in0=e_t[:, 2 * p:2 * p + 2, :],
                        in1=bass.AP(tensor=adj_sl.tensor, offset=adj_sl.offset,
                                    ap=[adj_sl.ap[0], [0, 2], adj_sl.ap[-1]]),
                        op=mult)
            elif poly:
                # ed = adj * (s + s^2/2)  [= adj*(e-1), PV_a folded in]
                nc.vector.tensor_scalar(out=pp[:], in0=s16[:], scalar1=0.5,
                                        scalar2=1.0, op0=mult, op1=add)
                nc.vector.tensor_tensor(out=pp2[:], in0=pp[:], in1=s16[:],
                                        op=mult)
                nc.vector.tensor_tensor(out=ed_t[:], in0=pp2[:], in1=adj_b,
                                        op=mult)
            else:
                nc.vector.tensor_tensor(out=ed_t[:], in0=e_t[:], in1=adj_b,
                                        op=mult)
            return ed_t

        def emit_pva(c, mt):
            nsl = slice(512 * c, 512 * (c + 1))
            x_h = x_h_of[c]
            last = mt == MT - 1
            for hh in range(4):
                nc.tensor.matmul(  # -= vT @ adj  (fp8 DoubleRow)
                    x_h[hh][0:65, :],
                    lhsT=vT8_sb[:, :, mt, 65 * hh:65 * (hh + 1)],
                    rhs=adj8_sb[:, :, mt, nsl],
                    start=False, stop=last, perf_mode=DR,
                )

        def emit_pv(c, mt, ed_t, skip_pva=False):
            nsl = slice(512 * c, 512 * (c + 1))
            x_h = x_h_of[c]
            poly = (c, mt) in POLY
            last = mt == MT - 1
            assert not (poly and last)
            if last:  # interleave so head h stops (and norms) earliest
                for hh in range(4):
                    nc.tensor.matmul(
                        x_h[hh][0:65, :],
                        lhsT=vT_sb[:, mt, 65 * hh:65 * (hh + 1)],
                        rhs=ed_t[:, hh, :],
                        start=False, stop=False,
                    )
                    nc.tensor.matmul(
                        x_h[hh][0:65, :],
                        lhsT=vT8_sb[:, :, mt, 65 * hh:65 * (hh + 1)],
                        rhs=adj8_sb[:, :, mt, nsl],
                        start=False, stop=True, perf_mode=DR,
                    )
                return
            for hh in range(4):
                nc.tensor.matmul(
                    x_h[hh][0:65, :],
                    lhsT=vT_sb[:, mt, 65 * hh:65 * (hh + 1)],
                    rhs=ed_t[:, hh, :],
                    start=(mt == 0), stop=False,
                )
            if not poly and not skip_pva:
                emit_pva(c, mt)

        def emit_accfin(c, heads=range(4)):
            # += [vsum_h; N] broadcast along n (plain accumulation)
            x_h = x_h_of[c]
            for hh in heads:
                nc.tensor.matmul(
                    x_h[hh][0:65, :],
                    lhsT=vsN_sb[0:1, hh, :],
                    rhs=ones512_sb[0:1, :],
                    start=False, stop=False,
                )

        norm_tiles = {}

        def emit_norm(c, heads=range(4)):
            # xsb = x * (1 / den); den = row 64 (already includes +N)
            x_h = x_h_of[c]
            if c not in norm_tiles:
                norm_tiles[c] = (
                    work.tile([1, 4, 512], f32, tag="den", name="den",
                              bufs=1),
                    work.tile([128, 4, 512], f32, tag="denb", name="den_b",
                              bufs=1),
                    work.tile([128, 2, 512], f32r, tag="xsb", name="xsb"),
                )
            denom_t, den_b, xsb = norm_tiles[c]
            xsb_of[c] = xsb
            for hh in heads:
                nc.vector.reciprocal(out=denom_t[0:1, hh, :],
                                     in_=x_h[hh][64:65, :])
                nc.gpsimd.partition_broadcast(
                    den_b[:, hh, :], denom_t[0:1, hh, :])
            for hh in heads:
                h2, kc = hh % 2, hh // 2
                nc.vector.tensor_tensor(
                    out=xsb[64 * h2:64 * h2 + 64, kc, :],
                    in0=x_h[hh][0:64, :],
                    in1=den_b[64 * h2:64 * h2 + 64, hh, :],
                    op=mult,
                )

        def emit_outproj(c, on_act=False):
            nsl = slice(512 * c, 512 * (c + 1))
            xsb = xsb_of[c]
            op_ps = psc.tile([128, 1024], f32, tag="sc", name="op_ps")
            out_t = work.tile([128, 1024], f32, tag="osb", name="out_t")
            for kc in range(2):
                for mtile in range(2):
                    nc.tensor.matmul(
                        op_ps[:, 512 * mtile:512 * (mtile + 1)],
                        lhsT=wm_sb[:, kc, 128 * mtile:128 * (mtile + 1)],
                        rhs=xsb[:, kc, :],
                        start=(kc == 0), stop=(kc == 1),
                    )
            for mtile in range(2):
                if on_act and mtile == 0:  # tail: split across ACT+DVE
                    nc.scalar.activation(
                        out=out_t[:, 512 * mtile:512 * (mtile + 1)],
                        in_=op_ps[:, 512 * mtile:512 * (mtile + 1)],
                        func=Ident, bias=bcol_sb[:, 4 + mtile:5 + mtile],
                        scale=1.0,
                    )
                else:
                    nc.vector.tensor_scalar(
                        out=out_t[:, 512 * mtile:512 * (mtile + 1)],
                        in0=op_ps[:, 512 * mtile:512 * (mtile + 1)],
                        scalar1=bcol_sb[:, 4 + mtile:5 + mtile],
                        scalar2=None, op0=add,
                    )
                nc.sync.dma_start(
                    out=out[128 * mtile:128 * (mtile + 1), nsl],
                    in_=out_t[:, 512 * mtile:512 * (mtile + 1)])

        for c in range(NCH):
            x_h_of[c] = [pacc.tile([128, 512], f32, tag=f"x{hh}",
                                   name=f"x{hh}") for hh in range(4)]
        tiles = [(c, mt) for c in range(NCH) for mt in range(MT)]
        emit_qk(*tiles[0], nodr=True)
        emit_qk(*tiles[1], nodr=True)
        emit_vproj_all()
        emit_vsum()
        NODR = 4
        # the first tiles' adj-subtract matmuls are deferred until vT8 and
        # adj8 have surely landed (accumulation is order-independent)
        PVA_DEFER = 4
        for i, (c, mt) in enumerate(tiles):
            if i + 2 < len(tiles):
                emit_qk(*tiles[i + 2], nodr=i + 2 < NODR)
            if i == PVA_DEFER:
                for (dc, dmt) in tiles[:PVA_DEFER]:
                    emit_pva(dc, dmt)
            ed_t = emit_ew(c, mt)
            emit_pv(c, mt, ed_t, skip_pva=i < PVA_DEFER)
            if 7 <= mt <= 10:
                emit_accfin(c, heads=[mt - 7])
            if c > 0 and mt in (2, 5, 8, 11):
                emit_norm(c - 1, heads=[(mt - 2) // 3])  # spread per head
            if c > 0 and mt == 13:
                emit_outproj(c - 1)    # previous chunk's projection
        # keep the PE clock warm through the final norm window so the
        # out-projection runs at full p-state (dummy QK-shaped matmuls)
        warm_ps = psc.tile([128, 1024], f32, tag="sc", name="warm_ps")
        for j in range(8):
            nc.tensor.matmul(
                warm_ps[:, 512 * (j % 2):512 * (j % 2 + 1)],
                lhsT=k8_sb[:, 0, :, 0, 0:128],
                rhs=q8_sb[:, 0, :, 0, 0:512],
                start=True, stop=True, perf_mode=DR,
            )
        emit_norm(NCH - 1)
        emit_outproj(NCH - 1, on_act=True)

    nc.compile()
    return nc


def host_prep(query, key, value, edges, Wq, bq, Wk, bk, Wv, bv, Wm, bm,
              N_=N, NH_=NH, B_=B):
    """Returns per-core input maps."""
    f32 = np.float32
    fp8 = ml_dtypes.float8_e4m3
    query = np.asarray(query, f32)
    key = np.asarray(key, f32)
    value = np.asarray(value, f32)
    edges = np.asarray(edges)
    Wq, bq = np.asarray(Wq, f32), np.asarray(bq, f32)
    Wk, bk = np.asarray(Wk, f32), np.asarray(bk, f32)
    Wv, bv = np.asarray(Wv, f32), np.asarray(bv, f32)
    Wm, bm = np.asarray(Wm, f32), np.asarray(bm, f32)
    MT = N_ // 128

    # head-major permutation: dd = h*DIM + dl  <->  o = dl*H + h
    dd = np.arange(D)
    perm = (dd % DIM) * H + (dd // DIM)

    def w8_layout(WT):  # WT [256(K=d_in), 256(dd)] -> [128, 2, 256] fp8
        return np.ascontiguousarray(
            WT.reshape(2, 128, D).transpose(1, 0, 2)).astype(fp8)

    def w32_layout(WT):
        return np.ascontiguousarray(WT.reshape(2, 128, D).transpose(1, 0, 2))

    wq8 = w8_layout((Wq[perm, :] * (ALPHA * SCALE)).T)
    wk8 = w8_layout((Wk[perm, :] * ALPHA).T)
    wv8 = w8_layout((Wv[perm, :] * ALPHA).T)
    w8pack_dev = np.ascontiguousarray(np.concatenate([wq8, wk8, wv8], axis=2))
    wm_dev = w32_layout(Wm[:, perm].T)
    bq8 = np.ascontiguousarray((bq[perm] * (ALPHA * SCALE)).reshape(2, 128).T)
    bk8 = np.ascontiguousarray((bk[perm] * ALPHA).reshape(2, 128).T)
    bm_col = np.ascontiguousarray(bm.reshape(2, 128).T)
    bcol_dev = np.ascontiguousarray(
        np.concatenate([bq8, bk8, bm_col], axis=1))
    brow16_dev = np.ascontiguousarray(
        (bv[perm] * ALPHA).reshape(1, D)).astype(ml_dtypes.bfloat16)

    def fold_x(x):  # [256, n] -> [128, 2, n] fp8
        return np.ascontiguousarray(
            x.reshape(2, 128, x.shape[1]).transpose(1, 0, 2)).astype(fp8)

    in_maps = []
    ncores = 2 * B_
    for c in range(ncores):
        b, half = c // 2, c % 2
        ns = slice(half * NH_, (half + 1) * NH_)
        adj = np.zeros((N_, N_), f32)
        np.add.at(adj, (edges[b, 0].astype(np.int64),
                        edges[b, 1].astype(np.int64)), 1.0)
        adjT_c = np.ascontiguousarray(adj[ns, :].T)
        # fp8 negated, m-folded: [64, 2slot, mt, n]
        adjT8n_c = np.ascontiguousarray(
            (-adjT_c).reshape(MT, 2, 64, NH_).transpose(2, 1, 0, 3)
        ).astype(fp8)
        in_maps.append({
            "xq8": fold_x(query[b][:, ns]),
            "xk8": fold_x(key[b]),
            "xv8": fold_x(value[b]),
            "w8pack": w8pack_dev, "wm32": wm_dev,
            "bcol": bcol_dev, "brow16": brow16_dev,
            "adjT": adjT_c.astype(ml_dtypes.bfloat16),
            "adjT8n": adjT8n_c,
        })
    return in_maps


LAST_RESULTS = None
LAST_NC = None


def kernel(**inputs):
    global LAST_RESULTS, LAST_NC
    from concourse.bass_utils import run_bass_kernel_spmd

    in_maps = host_prep(**inputs)
    nc = build_nc()
    LAST_NC = nc
    trace = bool(int(os.environ.get("KERNEL_TRACE", "0")))
    res = run_bass_kernel_spmd(nc, in_maps, core_ids=list(range(NCORES)),
                               trace=trace)
    LAST_RESULTS = res
    out = np.empty((B, D, N), np.float32)
    for c in range(NCORES):
        b, half = c // 2, c % 2
        out[b][:, half * NH:(half + 1) * NH] = res.results[c]["out"]
    return out
